# revision 68
# baseline (speedup 1.0000x reference)
"""Trainium2 Bass kernel for the bidirectional RNN language model.

Model (see problem reference): for a [L=128, B=32] int token grid,
  - forward + backward tanh-RNN (HID=20) over EMB=80 embeddings (VOCAB=32000)
  - per position: logits = [h_fwd[i], h_bwd[i+1]] @ h2o   -> [*, 32000]
  - output log_softmax(logits)  ->  [128, 32, 32000] f32  (512 MB)

Strategy: data-parallel over batch across 8 NeuronCores (4 batch columns
per core), no collectives. The 64 MB/core output write (~182 us at the
360 B/ns DMA roofline) is the hard floor; everything else minimizes
time-to-first-output-byte and keeps the output stream gap-free:
  - embeddings are pre-gathered on the HOST into the exact device operand
    layout (we[input] is pure data movement), with i2h / bias / h0 packed
    into the head of tile 0's block - so the kernel has no indirect DMAs
    / PE transposes and ONE small leading DMA unblocks recurrence step 0
    at ~3.2 us (HWDGE descriptor generation is a single shared device;
    separate preamble DMAs would serialize ~0.6 us apiece),
  - combined fwd+bwd recurrence: step tau = one K=112 matmul + one tanh
    (644 ns/step: two cross-engine semaphore hops dominate),
  - tile 0 = center positions {48..79} is recurrence-complete at step 78
    (~54 us); its softmax-normalizer pass (sum of exp over the 32000
    logits per row) is SPLIT across two engines: ACT runs table-exp
    chunks (1024 wide, in-place PSUM + accum_out partial sums), DVE runs
    Schraudolph exp chunks in parallel (512 wide: one tensor_scalar
    computing int32(A*logit+B) - the bitcast of which IS exp to ~±3% -
    then a free-axis reduce of the bitcast; per-row sums of 32000 terms
    average the element error to ~0.2%, i.e. ~2e-3 absolute on ln).
    The split pass takes ~27 us vs ~40 ACT-serial, so output streaming
    starts at ~84 us instead of ~99,
  - position tiles are symmetric pairs {48-79},{32-47,80-95},
    {16-31,96-111},{0-15,112-127}; each later tile's (ACT-only) exp pass
    hides under the previous tile's 45.5 us output window. The 16-step
    tanh batches STRADDLE window boundaries (8-step wall + 8 steps woven
    into the previous exp phase) so the exp pass can start ~6 us into
    its window and ACT's budget closes,
  - ln(sum) via the exponent-bit affine trick (K1*float(bits)-K2,
    |err| <= 0.03 abs = 2.2e-3 rel vs 2e-2 tolerance) - no Ln, so the
    whole kernel stays in ONE activation-table set {Exp,Tanh,Copy,
    Identity} and never pays the 1.3 us table reload,
  - pass 2 recomputes logit chunks (f32r matmuls, contraction layout
    [52] = fwd 0:20 | zero band | bwd 32:52, host-padded h2o) and
    subtracts ln(sum) on DVE (ACT shares odd chunks in the last window)
    into staging groups (6 buffers). The output DMA drains only 11%
    faster than the subtract stream produces, so tile 0's groups ramp
    512-col x12 / 1K x8 / 2K.. - group-ready then grows strictly slower
    than the drain and the stream never gaps; each tile's early hcat
    halves copy out during the recurrence (DVE idle there; a mid-window
    insert of even ~1 us punches an equal gap into the output stream),
    and the last tile's tail is 1K/512 so the final transfer drains fast.
TileScheduler discipline: the scheduler is priority-greedy over READY
instructions, so every cross-stream rate is pinned with order-only
add_dep_helper edges: p1 matmuls behind p2 matmuls at ~1.74:1 (arrive
just under ACT's exp rate - they never stall PE's in-order queue), the
head pass's merged ACT/DVE matmul order chained explicitly, rec matmuls
behind the p2 stream (woven steps spaced 3.3 us so their tanh has
drained through ACT's exp backlog before the next matmul heads PE's
in-order queue), stats behind the subtract stream, hcat copies emitted
strictly before any p1 of their tile.
Cost-model exec: ~269 us/core; DMA gapless from ~84 us on.
"""

import contextlib

import numpy as np

import concourse.bacc as bacc
import concourse.tile as tile
from concourse import bass, mybir
from concourse.bass_utils import run_bass_kernel_spmd
from concourse.tile_rust import add_dep_helper

L = 128
B = 32
V = 32000
EMB = 80
HID = 20
KDIM = EMB + HID          # 100
# Device-side contraction layout: hidden rows at partitions 0:20, zero pad
# 20:32 (compute-engine APs must start 32-aligned), embeddings at 32:112.
EOFF = 32
KP = EOFF + EMB           # 112
H2 = 2 * HID              # 40
# Projection contraction layout: fwd hidden rows at partitions 0:20, zero
# band 20:32 (host-supplied), bwd hidden rows at 32:52 - so both hcat
# copies are legal compute APs (32-aligned starts, <=32 partitions).
KH = EOFF + HID           # 52
NCORES = 8
BL = B // NCORES          # 4 batch columns per core
R = L * BL                # 512 output rows per core
NT = 4                    # position tiles of 128 rows (32 positions)

CH = 1024                 # vocab chunk per pass-1 PSUM tile (2 banks)
NFULL = V // CH           # 31 full chunks
REM = V - NFULL * CH      # 256
P2W = 512                 # pass-2 chunk width (1 PSUM bank)
NP2 = (V + P2W - 1) // P2W  # 63 pass-2 chunks (last = 256)

# Window-tile normalizer passes: the stats cascade (stats_{t+1} =
# stats_t + wall + exp pass + woven tanhs) at ~47.7us/window overruns the
# 45.5us DMA window and becomes the critical chain of the whole tail.
# DVE has ~3us/window of slack beside the subtract stream, so 2 512-col
# chunks per pass go to DVE Schraudolph exp (each costs DVE ~1.25us and
# saves ACT ~0.59us) - compressing the cascade toward the DMA floor.
WND = 0                   # DVE Schraudolph chunks per window pass (512)
WNA = 32 - (WND + 1) // 2  # ACT chunks per window pass
WREM = V - (WNA - 1) * CH - WND * P2W  # final ACT chunk width (256 or 768)
WDVE0 = (WNA - 1) * CH + WREM  # start of the DVE cols
NVCW = WNA + WND          # sparts cols for window tiles

# Head (tile 0) normalizer pass: split the vocab between ACT table-exp
# chunks (1024 wide, ~1.18 us each) and DVE Schraudolph chunks (512 wide,
# ~1.44 us each) so the two engines finish together at ~29 us.
NA_FULL = 21              # full 1024-col ACT chunks
ACT_REM = 256             # final ACT chunk width
NA = NA_FULL + 1          # 22 ACT chunks, cols [0, 21760)
ND = 20                   # 512-col DVE chunks, cols [21760, 32000)
DVE0 = NA_FULL * CH + ACT_REM
NCOL0 = NA + ND           # sparts[0] columns
assert DVE0 + ND * P2W == V

# Output staging groups per tile, in pass-2 chunk counts (P2W cols each).
# Tile 0 ramps up from 512-col groups so the first bytes hit HBM right
# after the normalizer lands; the last tile ramps down so the final
# transfer (and the program tail behind it) drains fast; middle tiles use
# 2048-col groups with 6 staging buffers (~12 chunks of rotation
# lookahead, so the stage-recycle semaphore never stalls the subtract
# stream at window boundaries).
# Host-packed recurrence-operand block width per tile: tile 0 leads with
# 20 i2h columns + 1 bias column, then 256 step columns; tiles 1-3 are
# just 256 step columns (block stride stays EXT).
EXT = 277
CO = (21, 0, 0, 0)        # step-column offset inside each tile's block

GROUPS = [
    [1] * 12 + [2] * 8 + [4] * 8 + [3],
    [2, 2] + [4] * 14 + [3],
    [2, 2] + [4] * 14 + [3],
    [2, 2] + [4] * 14 + [2, 1],
]
for _g in GROUPS:
    assert sum(_g) == NP2
SGW = 2048                # max staging width (8 KB/partition)

# ln-approx constants: ln(x) ~= K1*float(bits(x)) - K2, |err| <= 0.0299.
LN_K1 = 8.262958405176314e-08   # ln2 / 2^23
LN_K2 = 87.99984328235631       # 127*ln2 - 0.02985

# Schraudolph exp: bitcast_f32(int32(A*x + B)) ~= exp(x), elementwise
# rel err in [-3.9%, +2.0%], ~mean-zero so 512-term partial sums land
# within ~0.5%. Feeds only the ln(sum) normalizer.
SCH_A = 12102203.161561485      # 2^23 / ln2
SCH_B = 1064866808.0            # 127*2^23 - 486408

# Emission-order deadlines (us since window start) used to merge the
# per-window instruction streams; only relative order matters.
DL_REC = 0.644            # recurrence step period
DL_P2 = 0.658             # DVE subtract pace per 512 chunk
DL_P1 = 1.03              # ACT exp pace per 1024 chunk
# head-pass deadlines
DL_HA = 1.33              # ACT chunk period in the split pass (incl tanhs)
DL_HD = 1.44              # DVE chunk period in the split pass

F32 = mybir.dt.float32
F32R = mybir.dt.float32r
I32 = mybir.dt.int32
AF = mybir.ActivationFunctionType
ALU = mybir.AluOpType
AXL = mybir.AxisListType

_CACHE = {}

# Optional extra kwargs for run_bass_kernel_spmd (used by test harness for
# tracing); harmless defaults for grading.
RUN_KWARGS = {}
LAST_RESULTS = None

# Symmetric position-tile pairs: tile pt = positions [a, a+16) u [b, b+16),
# ready after recurrence step max(b+15, 127-a) = 78/94/110/126.
PTS = [(48, 64), (32, 80), (16, 96), (0, 112)]


def _build():
    nc = bacc.Bacc("TRN2", debug=False, num_devices=NCORES)

    # emb: host-pregathered recurrence operands in the exact device layout,
    # one [KP, 277] block per position tile: rows 32:112 of cols 256k+8j+4h+b
    # hold we[token] for (step 32k+j, chain half h, batch b) (half 1 = the
    # backward chain, position 127-pos); rows 0:32 are host zeros except
    # tile 0's cols 0:8 = h0 broadcast. Block cols 256:276 carry i2h (tile 0
    # only) and col 276 rows 0:20 the tanh bias - so ONE DMA delivers
    # everything recurrence step 0 needs (HWDGE descriptor generation is a
    # single shared device; separate preamble DMAs would serialize ~0.6us
    # apiece before their 0.9us completion-semaphore latency even starts).
    emb_d = nc.dram_tensor("emb", [KP, NT * EXT], F32, kind="ExternalInput")
    # float32r: PE streams fp32 at full rate with tf32-like operand
    # truncation - ~2e-4 relative effect on logits, far inside tolerance.
    h2o_d = nc.dram_tensor("h2o", [KH, V], F32R, kind="ExternalInput")
    out_d = nc.dram_tensor("out", [R, V], F32, kind="ExternalOutput")

    with tile.TileContext(nc) as tc:
        with (
            tc.tile_pool(name="const", bufs=1) as const,
            tc.tile_pool(name="hbuf", bufs=1) as hbuf,
            tc.tile_pool(name="stat", bufs=1) as stat,
            tc.tile_pool(name="stage", bufs=6) as stage,
            tc.tile_pool(name="hsc", bufs=4) as hsc,
        ):
            # Warm the single activation table ({Exp,Tanh,Copy,Identity}
            # all live in exp_and_others) before the recurrence starts.
            warm = const.tile([1, 1], F32)
            nc.vector.memset(warm[:], 0.0)
            nc.scalar.activation(out=warm[:], in_=warm[:], func=AF.Exp)

            # Combined recurrence operand buffers: step tau's block (8 cols)
            # at tile tau//32, local cols CO[k]+8*(tau%32): [fwd tau |
            # bwd 127-tau]. Rows 0:20 = hidden inputs ([hiddenf[tau] |
            # hiddenb[128-tau]]), rows 20:32 zero pad (host-supplied),
            # rows 32:112 = embedding^T. Tile 0's block leads with i2h
            # (cols 0:20) and the tanh bias (col 20), then its step
            # columns at 21+8j - so one SMALL first DMA (cols 0:29, i.e.
            # weights + bias + step 0) unblocks recurrence step 0 ~0.3us
            # before the bulk of the block lands. The bulk DMA covers only
            # rows 20:112 of cols 29:277: rows 0:20 there are tanh outputs
            # (a full-height bulk DMA would WAW-order every early tanh
            # behind it).
            rhsC = [
                hbuf.tile(
                    [KP, EXT if k == 0 else 256],
                    F32, name=f"rhsC{k}", tag=f"rhsC{k}",
                )
                for k in range(NT)
            ]
            nc.sync.dma_start(out=rhsC[0][:, 0:29], in_=emb_d[:, 0:29])
            nc.sync.dma_start(
                out=rhsC[0][HID:, 29:EXT], in_=emb_d[HID:, 29:EXT]
            )
            for k in range(1, NT):
                nc.sync.dma_start(
                    out=rhsC[k][:, 0:256], in_=emb_d[:, EXT * k : EXT * k + 256]
                )
            i2h_sb = rhsC[0][:, 0:20]
            biasc = rhsC[0][0:HID, 20:21]

            h2o_sb = const.tile([KH, V], F32R)
            for q in range(4):
                nc.sync.dma_start(
                    out=h2o_sb[:, q * (V // 4) : (q + 1) * (V // 4)],
                    in_=h2o_d[:, q * (V // 4) : (q + 1) * (V // 4)],
                )

            hcatT = [
                hbuf.tile([KH, 128], F32R, name=f"hcatT{k}", tag=f"hcatT{k}")
                for k in range(NT)
            ]
            # zero the 20:32 band (and, harmlessly, the fwd rows before
            # their copies land): matmul contracts over all 52 partitions,
            # and 0 * 0 from both zero bands contributes nothing. Memset on
            # an f32r region fails codegen, so zero an f32 scratch and copy
            # it in (bitcast), matching the other f32r-producing copies.
            zscr = const.tile([EOFF, 128], F32)
            nc.vector.memset(zscr[:], 0.0)
            for k in range(NT):
                nc.vector.tensor_copy(
                    out=hcatT[k][0:EOFF, :], in_=zscr[:].bitcast(F32R)
                )
            sparts = [
                stat.tile(
                    [128, NCOL0 if k == 0 else NVCW],
                    F32, name=f"sparts{k}", tag=f"sparts{k}",
                )
                for k in range(NT)
            ]
            logs = [
                stat.tile([128, 1], F32, name=f"logs{k}", tag=f"logs{k}")
                for k in range(NT)
            ]
            neg_logs = [
                stat.tile([128, 1], F32, name=f"nlog{k}", tag=f"nlog{k}")
                for k in range(NT)
            ]

            # PSUM budget is 8 banks: rec 1, p1 2x1024 = 4, then either
            # p2 3x512 (deeper subtract rotation absorbs PE stalls from
            # the tanh-wall matmuls) or p2 2x512 + 1 window-Schraudolph.
            with contextlib.ExitStack() as _stack:
                rps = _stack.enter_context(
                    tc.tile_pool(name="rps", bufs=1, space="PSUM"))
                p1ps = _stack.enter_context(
                    tc.tile_pool(name="p1ps", bufs=2, space="PSUM"))
                p2ps = _stack.enter_context(
                    tc.tile_pool(name="p2ps", bufs=3 if WND == 0 else 2,
                                 space="PSUM"))
                ws = (
                    _stack.enter_context(
                        tc.tile_pool(name="ws", bufs=1, space="PSUM"))
                    if WND else p2ps
                )

                def emit_rec(step, after=None, chain=False):
                    k0 = step // 32
                    c0 = CO[k0] + 8 * (step % 32)
                    pc = rps.tile([HID, 2 * BL], F32, tag="rec")
                    st["rec"] = nc.tensor.matmul(
                        out=pc[:],
                        lhsT=i2h_sb,
                        rhs=rhsC[k0][:, c0 : c0 + 8],
                        start=True,
                        stop=True,
                    )
                    if chain:
                        chain_mm(st["rec"])
                    elif after is not None:
                        # pace the recurrence tail behind the pass-2 stream
                        # so the scheduler can't freeze all rec steps ahead
                        # of the output-feeding matmuls.
                        add_dep_helper(
                            st["rec"].ins, after.ins, sync=False,
                            reason="rec behind p2 stream",
                        )
                    t1 = step + 1
                    k1 = t1 // 32
                    c1 = CO[k1] + 8 * (t1 % 32)
                    nc.scalar.activation(
                        out=rhsC[k1][0:HID, c1 : c1 + 8],
                        in_=pc[:],
                        func=AF.Tanh,
                        bias=biasc,
                    )

                def emit_hcat_fwd(pt, s):
                    # fwd rows (partitions 0:20) of pair-half s: positions
                    # p0..p0+15, sources final after rec step p0+15-1.
                    p0 = PTS[pt][s]
                    d0 = 64 * s
                    kf = p0 // 32
                    fc0 = CO[kf] + 8 * (p0 % 32)
                    tf = rhsC[kf]
                    src_f = bass.AP(
                        tensor=tf.tensor,
                        offset=tf.offset + fc0,
                        ap=[[tf.ap[0][0], HID], [8, 16], [1, 4]],
                    ).bitcast(F32R)
                    nc.vector.tensor_copy(
                        out=hcatT[pt][0:HID, d0 : d0 + 64], in_=src_f
                    )

                def emit_hcat_bwd(pt, s):
                    # bwd rows (partitions 32:52): hiddenb[p0+1..p0+16],
                    # sources final after rec step 126-p0.
                    p0 = PTS[pt][s]
                    d0 = 64 * s
                    b_hi = 127 - p0
                    kb = b_hi // 32
                    bc0 = CO[kb] + 8 * (b_hi % 32) + 4
                    tb = rhsC[kb]
                    src_b = bass.AP(
                        tensor=tb.tensor,
                        offset=tb.offset + bc0,
                        ap=[[tb.ap[0][0], HID], [-8, 16], [1, 4]],
                    ).bitcast(F32R)
                    nc.vector.tensor_copy(
                        out=hcatT[pt][EOFF : EOFF + HID, d0 : d0 + 64],
                        in_=src_b,
                    )

                def emit_hcat_early(pt):
                    # halves whose sources are final 16 steps before the
                    # tile's ready step: fwd of the low pair, bwd of the
                    # high pair.
                    emit_hcat_fwd(pt, 0)
                    emit_hcat_bwd(pt, 1)

                def emit_hcat_late(pt):
                    emit_hcat_bwd(pt, 0)
                    emit_hcat_fwd(pt, 1)

                def chain_mm(mm):
                    # Pin PE stream order for the head pass: the scheduler
                    # is priority-greedy and would otherwise run one
                    # engine's whole matmul stream first, starving the
                    # other engine through PE's in-order queue.
                    if st.get("lastmm") is not None:
                        add_dep_helper(
                            mm.ins, st["lastmm"].ins, sync=False,
                            reason="head pass PE order",
                        )
                    st["lastmm"] = mm

                def emit_p1(pt, v0, w, col, after=None, chain=False):
                    # ACT pass-1 chunk: logits to PSUM, exp in place,
                    # partial sum into `col` via accum_out.
                    p1t = p1ps.tile([128, CH], F32, tag="p1", name="p1t")
                    for m in range(0, w, 512):
                        mw = min(512, w - m)
                        mm = nc.tensor.matmul(
                            out=p1t[:, m : m + mw],
                            lhsT=hcatT[pt][:],
                            rhs=h2o_sb[:, v0 + m : v0 + m + mw],
                            start=True,
                            stop=True,
                        )
                        if after is not None:
                            # Pin PE order: keep this pass-1 matmul behind
                            # the paired pass-2 matmul so the scheduler
                            # can't starve the DVE/DMA stream by hoisting
                            # P1 work.
                            add_dep_helper(
                                mm.ins, after.ins, sync=False,
                                reason="wave interleave order",
                            )
                            after = None
                        if chain:
                            chain_mm(mm)
                        st["p1mm"] = mm
                    nc.scalar.activation(
                        out=p1t[:, :w], in_=p1t[:, :w], func=AF.Exp,
                        accum_out=col,
                    )

                def emit_p1_dve(pt, v0, w, col, pool, tag, chain=False,
                                after=None, flush=True):
                    # DVE pass-1 chunk: Schraudolph exp. int32(A*logit + B)
                    # into a pairing buffer; the bitcast-f32 reduce runs
                    # once per PAIR (flush) to amortize the op overhead.
                    p2t = pool.tile([128, P2W], F32, tag=tag, name=tag)
                    mm = nc.tensor.matmul(
                        out=p2t[:, :w],
                        lhsT=hcatT[pt][:],
                        rhs=h2o_sb[:, v0 : v0 + w],
                        start=True,
                        stop=True,
                    )
                    if chain:
                        chain_mm(mm)
                    elif after is not None:
                        add_dep_helper(
                            mm.ins, after.ins, sync=False,
                            reason="window dve-exp order",
                        )
                    st["p1mm"] = mm
                    if st.get("hpair") is None:
                        hi = hsc.tile([128, 2 * P2W], I32, tag="hsc",
                                      name="hsct")
                        off = 0
                    else:
                        hi, off = st["hpair"]
                    nc.vector.tensor_scalar(
                        out=hi[:, off : off + w], in0=p2t[:, :w],
                        scalar1=SCH_A, scalar2=SCH_B,
                        op0=ALU.mult, op1=ALU.add,
                    )
                    if flush or off + w >= 2 * P2W:
                        nc.vector.tensor_reduce(
                            out=col, in_=hi[:, 0 : off + w].bitcast(F32),
                            axis=AXL.X, op=ALU.add,
                        )
                        st["hpair"] = None
                    else:
                        st["hpair"] = (hi, off + w)

                def emit_stats(pt, after=None):
                    # logs[pt] = ln(sum(sparts[pt])) without the Ln table:
                    # exponent-bit affine approx, |err| <= 0.0299 absolute
                    # (2.2e-3 relative on the output, tolerance is 2e-2).
                    # Two DVE ops, no cross-engine roundtrip on the gate.
                    ncol = NCOL0 if pt == 0 else NVCW
                    s_t = stat.tile([128, 1], F32, name=f"s{pt}", tag=f"s{pt}")
                    rd = nc.vector.tensor_reduce(
                        out=s_t[:], in_=sparts[pt][:, 0:ncol],
                        axis=AXL.X, op=ALU.add,
                    )
                    if after is not None:
                        # keep the stats chain behind the current tile's
                        # subtract stream on DVE - the scheduler would
                        # otherwise hoist it (and its blocking wait).
                        add_dep_helper(
                            rd.ins, after.ins, sync=False,
                            reason="stats after subtract stream",
                        )
                    # one op: int32-typed input AP converts on read, so
                    # the exponent-bit ln affine needs no separate copy
                    nc.vector.tensor_scalar(
                        out=logs[pt][:], in0=s_t[:].bitcast(I32),
                        scalar1=LN_K1, scalar2=-LN_K2,
                        op0=ALU.mult, op1=ALU.add,
                    )
                    if pt == NT - 1:
                        nc.vector.tensor_scalar(
                            out=neg_logs[pt][:], in0=logs[pt][:],
                            scalar1=-1.0, scalar2=None, op0=ALU.mult,
                        )

                # staging state for the output groups of the current tile
                st = {"stg": None, "off": 0, "g0": 0, "gi": 0,
                      "groups": GROUPS[0]}

                def emit_p2(pt, j, share_act=False):
                    v0 = j * P2W
                    w = P2W if j < NP2 - 1 else V - v0
                    p2t = p2ps.tile([128, P2W], F32, tag="p2", name="p2t")
                    st["mm"] = nc.tensor.matmul(
                        out=p2t[:, :w],
                        lhsT=hcatT[pt][:],
                        rhs=h2o_sb[:, v0 : v0 + w],
                        start=True,
                        stop=True,
                    )
                    if st["off"] == 0:
                        st["stg"] = stage.tile(
                            [128, SGW], F32, tag="stg", name="stg"
                        )
                        st["g0"] = v0
                    off = st["off"]
                    if share_act:
                        sub = nc.scalar.activation(
                            out=st["stg"][:, off : off + w],
                            in_=p2t[:, :w],
                            func=AF.Identity,
                            bias=neg_logs[pt][:],
                        )
                    else:
                        sub = nc.vector.tensor_scalar(
                            out=st["stg"][:, off : off + w],
                            in0=p2t[:, :w],
                            scalar1=logs[pt][:],
                            scalar2=None,
                            op0=ALU.subtract,
                        )
                    st["sub"] = sub
                    st["off"] = off + w
                    # close the group when its chunk count is reached
                    gend = sum(st["groups"][: st["gi"] + 1])
                    if j + 1 == gend:
                        gw = st["off"]
                        r0a, r0b = 4 * PTS[pt][0], 4 * PTS[pt][1]
                        dst = bass.AP(
                            tensor=out_d,
                            offset=r0a * V + st["g0"],
                            ap=[[(r0b - r0a) * V, 2], [V, 64], [1, gw]],
                        )
                        nc.sync.dma_start(out=dst, in_=st["stg"][:, :gw])
                        st["off"] = 0
                        st["gi"] += 1

                # --- prefix: recurrence steps 0..78. Everything it needs
                # (host-layout embeddings, i2h, bias, h0) is already in
                # flight; every tile's early hcat halves copy out as soon
                # as their sources are final (DVE idles through the whole
                # recurrence, so these are free here - and inserting them
                # into a window's subtract stream instead would stall the
                # output DMA, which runs right on the stream's heels).
                for step in range(79):
                    emit_rec(step)
                    if step == 16:
                        emit_hcat_early(3)
                    elif step == 31:
                        emit_hcat_early(2)
                    elif step == 47:
                        emit_hcat_early(1)
                    elif step == 62:
                        emit_hcat_early(0)

                # --- tile 0 normalizer phase, split across ACT and DVE;
                # recurrence steps 79..86 (needed only for tile 2) weave
                # between chunks so each window's tanh wall shrinks to 8
                # steps and its exp phase can start ~4 us earlier.
                emit_hcat_late(0)
                # Deadlines reflect true engine cadences: ACT 1.184/chunk
                # plus the 8 woven tanh slots (~0.38 each) -> ~1.33
                # effective; DVE 1.44/chunk. The merged order is pinned on
                # PE via chain_mm.
                ev = []
                for i in range(NA):
                    w = CH if i < NA_FULL else ACT_REM
                    ev.append((DL_HA * i, 1, "ha", (i * CH, w, i)))
                for j in range(ND):
                    ev.append((DL_HD * j + 0.10, 1, "hd",
                               (DVE0 + j * P2W, P2W, j)))
                for k in range(8):
                    ev.append((3.6 * k + 0.5, 0, "rec", 79 + k))
                ev.sort(key=lambda e: (e[0], e[1]))
                st["lastmm"] = None
                for _, _, kind, a in ev:
                    if kind == "rec":
                        emit_rec(a, chain=True)
                    elif kind == "ha":
                        v0, w, i = a
                        emit_p1(0, v0, w, sparts[0][:, i : i + 1], chain=True)
                    else:
                        v0, w, j = a
                        ci = NA + j
                        emit_p1_dve(
                            0, v0, w, sparts[0][:, ci : ci + 1],
                            p2ps, "p2", chain=True,
                        )
                st["lastmm"] = None
                emit_stats(0)

                # --- four output windows; window pt streams tile pt while
                # the next tile's recurrence tail + exp pass run under it.
                for pt in range(NT):
                    last = pt == NT - 1
                    st["groups"] = GROUPS[pt]
                    st["gi"] = 0
                    st["off"] = 0
                    ev = []
                    if not last:
                        base = 87 + 16 * pt
                        for i in range(8):   # tanh wall: steps base..base+7
                            ev.append((DL_REC * i, 0, "rec", base + i))
                        for i in range(8):   # woven: steps base+8..base+15
                            # 3.3us spacing: the step's tanh must already
                            # have drained through ACT's exp backlog when
                            # the next step's matmul reaches the head of
                            # PE's in-order queue - a stalled rec matmul
                            # there blocks the p2 stream behind it and
                            # punches gaps into the subtract stream.
                            s2 = base + 8 + i
                            if s2 < L - 1:
                                ev.append((8.0 + 3.3 * i, 1, "rec", s2))
                        # hcat halves MUST be emitted before any p1 of
                        # their tile: a pass-1 matmul emitted ahead of the
                        # copies would legitimately read the stale
                        # pre-copy hcatT (program-order read-before-write).
                        # The early halves' sources were final 16 steps
                        # ago; emit them at window start so DVE does them
                        # in its first slack.
                        ev.append((DL_P2 * 8 - 0.45, 0, "hcat_l", pt + 1))
                        for k in range(WNA):
                            # pace pass-1 matmuls at ~1.74 pass-2 chunks
                            # per chunk: they then arrive just below ACT's
                            # exp rate, never stall on the PSUM rotation,
                            # and so never block the pass-2 stream behind
                            # them in PE's in-order queue.
                            j = min(8 + int(1.74 * k), NP2 - 1)
                            ev.append(
                                (DL_P2 * j - 0.35, 1, "p1", (k, j))
                            )
                        for d in range(WND):
                            # the DVE Schraudolph chunks ride DVE's slack
                            # LATE in the window, where the subtract stream
                            # has built enough lead over the output DMA to
                            # absorb the 1.25us insert without a gap
                            j = 36 + 16 * d
                            ev.append((DL_P2 * j - 0.30, 1, "p1d", (d, j)))
                    for j in range(NP2):
                        ev.append((max(DL_P2 * j - 0.4, 0.05), 2, "p2", j))
                    ev.sort(key=lambda e: (e[0], e[1]))
                    p2mm = {}
                    for _, _, kind, a in ev:
                        if kind == "rec":
                            emit_rec(a, after=st.get("mm"))
                        elif kind == "hcat_e":
                            emit_hcat_early(a)
                        elif kind == "hcat_l":
                            emit_hcat_late(a)
                        elif kind == "p1":
                            k, j = a
                            w = CH if k < WNA - 1 else WREM
                            emit_p1(pt + 1, k * CH, w,
                                    sparts[pt + 1][:, k : k + 1],
                                    after=p2mm.get(j, st.get("mm")))
                        elif kind == "p1d":
                            d, j = a
                            emit_p1_dve(
                                pt + 1, WDVE0 + d * P2W, P2W,
                                sparts[pt + 1][:, WNA + d : WNA + d + 1],
                                ws, "wsd",
                                after=p2mm.get(j, st.get("mm")),
                            )
                        else:
                            emit_p2(pt, a, share_act=last and (a % 2 == 1))
                            p2mm[a] = st["mm"]
                    if not last:
                        emit_stats(pt + 1, after=st["sub"])

    nc.compile()
    return nc


def _get_nc():
    if "nc" not in _CACHE:
        _CACHE["nc"] = _build()
    return _CACHE["nc"]


def kernel(input, we, i2h, h2o, bias, h0):
    global LAST_RESULTS
    input = np.asarray(input)
    we = np.ascontiguousarray(np.asarray(we), dtype=np.float32)
    i2h = np.ascontiguousarray(np.asarray(i2h), dtype=np.float32)
    h2o = np.asarray(h2o, dtype=np.float32)
    h2o_dev = np.zeros((KH, V), dtype=np.float32)
    h2o_dev[0:HID] = h2o[0:HID]
    h2o_dev[EOFF:] = h2o[HID:]
    bias = np.asarray(bias, dtype=np.float32)
    h0 = np.asarray(h0, dtype=np.float32)

    biasc = np.ascontiguousarray(bias.reshape(1, HID).T)          # [20, 1]
    h0r = np.ascontiguousarray(
        np.repeat(h0.reshape(1, HID).T, 2 * BL, axis=1)           # [20, 8]
    )
    # Reorder i2h into the padded device contraction layout: hidden-state
    # weight rows first, zeros, then embedding weight rows.
    i2h_dev = np.zeros((KP, HID), dtype=np.float32)
    i2h_dev[0:HID] = i2h[EMB:]
    i2h_dev[EOFF:] = i2h[0:EMB]

    nc = _get_nc()
    in_maps = []
    for c in range(NCORES):
        tok = input[:, BL * c : BL * (c + 1)].astype(np.int64)    # [L, BL]
        # Host-side embedding gather straight into the device operand
        # layout (see emb_d in _build): per tile k a [KP, EXT] block whose
        # col 8j+4h+b = (step 32k+j, half h, batch b); half 1 = the
        # backward chain (position 127-pos). Zero band rows 0:32, h0 in
        # tile 0 cols 0:8, i2h in tile 0 cols 256:276, bias in col 276.
        pair = np.stack([we[tok], we[tok[::-1]]], axis=1)  # [L, 2, BL, EMB]
        embT = pair.transpose(3, 0, 1, 2).reshape(EMB, 2 * R)
        emb_all = np.zeros((KP, NT * EXT), dtype=np.float32)
        for k in range(NT):
            c0 = EXT * k + CO[k]
            emb_all[EOFF:, c0 : c0 + 256] = embT[:, 256 * k : 256 * (k + 1)]
        emb_all[:, 0:20] = i2h_dev
        emb_all[0:HID, 20] = biasc[:, 0]
        emb_all[0:HID, 21:29] = h0r
        in_maps.append({"emb": emb_all, "h2o": h2o_dev})

    res = run_bass_kernel_spmd(
        nc, in_maps, core_ids=list(range(NCORES)), **RUN_KWARGS
    )
    LAST_RESULTS = res
    parts = [res.results[c]["out"].reshape(L, BL, V) for c in range(NCORES)]
    return np.concatenate(parts, axis=1)
